# revision 25
# baseline (speedup 1.0000x reference)
"""CRF log-partition kernel for Trainium2 (8 NeuronCores, data-parallel batch).

Algorithm: the reference forward scan
    alpha' = logsumexp(alpha[None,:] + trans, axis=prev) + emit
is linearized to probability space:
    p' = (M @ p) * E,   M = exp(trans), E = exp(emit) * 2^-7
(the 2^-7 cancels the mean per-step log-growth of ~4.85, so the state
stays in f32/bf16 range with no renormalization; all scale bookkeeping
is recovered on the host from state snapshots).

Each batch item's 2048-step sequence is split into P=64 segments of L=32
steps scanned in parallel (products of positive matrices converge to
rank-1, so each segment's output direction is independent of its init;
scales are fixed up by an 8-tick prefix-correction pass seeded with the
previous segment's final state). Per core: 32 batch x 64 segments = 2048
chains laid out as X[128, 1024] bf16 — tag-block A (chains 0-1023) on
partitions 0-47, block B (chains 1024-2047) on partitions 64-111 (junk
rows are killed by zero rows of the padded [112,128] lhsT).

E = exp(emissions)*2^-7 is precomputed on the HOST in bf16, already
transposed to the on-chip layout [tick, tag-slot(128), chain(1024)], so
the device pipeline is just: DMA -> per-tick (PE matmul, DVE/Pool
elementwise multiply). No on-device exp, no PE transposes, no
PSUM->SBUF staging copies. Per tick the 1024 columns are split into
independent column-groups, each its own serial matmul->multiply chain
(staggered to hide the PE->DVE/Pool semaphore+PSUM-access latency);
groups are assigned to DVE and Pool to balance engine busy time.

Host stitches per-(batch,segment) log-scales in float64 from three bf16
snapshots per core: pass-1 state at tick PREFIX, pass-1 final, pass-2
final.

mask does not affect the forward value (m*x + (1-m)*x == x) and is ignored.
"""

import math

import numpy as np

B, S, T = 256, 2048, 48
NEG = -10000.0
NCORES = 8
BC = B // NCORES          # batch per core = 32
P = 64                    # segments per batch item
L = S // P                # ticks per segment = 32
PREFIX = 2                # prefix-correction ticks
C2POW = -7                # constant rescale folded into E
COLS = BC * P // 2        # chains per block (columns of X) = 1024
ROWS = 112                # meaningful partition rows (blocks at 0-47 / 64-111)
XROWS = 128               # physical tile rows
TPG = 1                   # ticks per resident eT tile (one DMA each)
QBUFS = 2                 # PSUM q bufs per column-group
XBUFS = 8
# per-tick column-groups: (engine, col0, col1); "v"=DVE, "p"=Pool
# "v": DVE multiplies straight from PSUM; "p": ACT copies q to SBUF bf16
# (GPSIMD cannot read PSUM), then Pool multiplies all-SBUF.
MGROUPS = (("v", 0, 512), ("v", 512, 1024))
PSKEW = 2                 # p-group issue lag (ticks) to avoid PE head-of-line

_CACHE = {}


def _build(**cfg):
    g = globals()
    saved = {k: g[k] for k in cfg}
    g.update(cfg)
    try:
        return _build_inner()
    finally:
        g.update(saved)


def _build_inner():
    from contextlib import ExitStack

    import concourse.bacc as bacc
    import concourse.bass as bass
    import concourse.mybir as mybir
    import concourse.tile as tile

    f32 = mybir.dt.float32
    bf16 = mybir.dt.bfloat16

    nc = bacc.Bacc(None, target_bir_lowering=False)

    NTILE = L // TPG
    eet_d = nc.dram_tensor("eet", [L, ROWS, COLS], bf16, kind="ExternalInput")
    w_d = nc.dram_tensor("wlhs", [ROWS, XROWS], bf16, kind="ExternalInput")
    pinit_d = nc.dram_tensor("pinit", [48, 32], bf16, kind="ExternalInput")
    snap16_d = nc.dram_tensor("snap16", [ROWS, COLS], bf16, kind="ExternalOutput")
    snapf_d = nc.dram_tensor("snapf", [ROWS, COLS], bf16, kind="ExternalOutput")
    snap2_d = nc.dram_tensor("snap2", [ROWS, COLS], bf16, kind="ExternalOutput")

    with tile.TileContext(nc) as tc:
        with ExitStack() as ctx:
            consts = ctx.enter_context(tc.tile_pool(name="consts", bufs=1))
            epool = ctx.enter_context(tc.tile_pool(name="epool", bufs=NTILE))
            xpool = ctx.enter_context(tc.tile_pool(name="xpool", bufs=XBUFS))
            qpool = ctx.enter_context(
                tc.tile_pool(name="qpool", bufs=QBUFS, space=bass.MemorySpace.PSUM))
            qbpool = ctx.enter_context(tc.tile_pool(name="qbpool", bufs=QBUFS))

            w_sb = consts.tile([ROWS, XROWS], bf16, tag="w")
            nc.sync.dma_start(w_sb[:], w_d[:])

            NG = len(MGROUPS)
            GW = [c1 - c0 for (_, c0, c1) in MGROUPS]
            lag = [0 if kind == "v" else PSKEW for (kind, _, _) in MGROUPS]

            # init per-group X first: the tiny pinit DMA must not queue
            # behind the eT loads
            xg = []
            for gi in range(NG):
                xt = xpool.tile([XROWS, GW[gi]], bf16, tag=f"x{gi}",
                                name=f"x{gi}")
                nc.gpsimd.memset(xt[:], 1.0)
                xg.append(xt)
            nc.sync.dma_start(xg[0][0:48, 0:32], pinit_d[:])  # cols 0-31 in g0

            # resident E tiles: [128, TPG*COLS], free = (tick, chain)
            etiles = []
            for i in range(NTILE):
                et = epool.tile([XROWS, TPG * COLS], bf16, tag="e",
                                name=f"e{i}")
                src = eet_d[i * TPG:(i + 1) * TPG].rearrange("t p c -> p t c")
                dma_eng = nc.scalar if i % 2 else nc.sync
                dma_eng.dma_start(et[0:ROWS, :], src)
                etiles.append(et)

            def tick_g(gi, x_in, k, matmul):
                kind, c0, c1 = MGROUPS[gi]
                base = (k % TPG) * COLS
                esl = etiles[k // TPG][:, base + c0:base + c1]
                x_out = xpool.tile([XROWS, GW[gi]], bf16, tag=f"x{gi}",
                                   name=f"x{gi}_{k}")
                eng = nc.vector if kind == "v" else nc.gpsimd
                if not matmul:
                    eng.tensor_mul(x_out[:], x_in[:], esl)
                    return x_out
                q = qpool.tile([XROWS, GW[gi]], f32, tag=f"q{gi}",
                               name=f"q{gi}")
                nc.tensor.matmul(q[:], w_sb[:], x_in[0:ROWS, :])
                if kind == "v":
                    eng.tensor_mul(x_out[:], q[:], esl)
                else:
                    qb = qbpool.tile([XROWS, GW[gi]], bf16,
                                     tag=f"qb{gi}", name=f"qb{gi}")
                    nc.scalar.activation(
                        qb[:], q[:], mybir.ActivationFunctionType.Copy)
                    eng.tensor_mul(x_out[:], qb[:], esl)
                return x_out

            # pass 1; p-groups lag PSKEW ticks in issue order so their
            # slower chains never block the PE queue head for the v-groups
            for r in range(L + PSKEW):
                for gi, (kind, c0, c1) in enumerate(MGROUPS):
                    k = r - lag[gi]
                    if not 0 <= k < L:
                        continue
                    xg[gi] = tick_g(gi, xg[gi], k, matmul=(k > 0))
                    if k + 1 == PREFIX:
                        nc.sync.dma_start(snap16_d[0:ROWS, c0:c1],
                                          xg[gi][0:ROWS, :])
                    if k + 1 == L:
                        nc.sync.dma_start(snapf_d[0:ROWS, c0:c1],
                                          xg[gi][0:ROWS, :])

            # pass 2: init = pass-1 finals shifted by one 32-col segment slot
            x2g = []
            for gi in range(NG):
                w = GW[gi]
                x2 = xpool.tile([XROWS, w], bf16, tag=f"x{gi}",
                                name=f"x2_{gi}")
                if gi == 0:
                    nc.gpsimd.memset(x2[:, 0:32], 1.0)     # seg-0 slot unused
                    # block-B first seg init <- block-A last seg (A rows of
                    # the very last column slot, partition-shifted via DMA)
                    nc.sync.dma_start(
                        x2[64:112, 0:32], xg[NG - 1][0:48, GW[NG - 1] - 32:])
                else:
                    nc.vector.tensor_copy(
                        x2[:, 0:32], xg[gi - 1][:, GW[gi - 1] - 32:])
                nc.vector.tensor_copy(x2[:, 32:w], xg[gi][:, 0:w - 32])
                x2g.append(x2)
            for r in range(PREFIX + PSKEW):
                for gi, (kind, c0, c1) in enumerate(MGROUPS):
                    k = r - lag[gi]
                    if not 0 <= k < PREFIX:
                        continue
                    x2g[gi] = tick_g(gi, x2g[gi], k, matmul=True)
                    if k + 1 == PREFIX:
                        nc.sync.dma_start(snap2_d[0:ROWS, c0:c1],
                                          x2g[gi][0:ROWS, :])

    nc.compile()
    return nc


def _host_consts(transitions):
    """W lhsT, p_init (analytic first log-step), stitch constants."""
    import ml_dtypes

    tr = transitions.astype(np.float64)
    M = np.exp(tr)                                   # M[next, prev]
    wl = np.zeros((ROWS, XROWS), np.float64)
    wl[0:48, 0:48] = M.T                             # lhsT[k, m] = M[m, k]
    wl[64:112, 64:112] = M.T

    # analytic first step: v[next] = logsumexp_prev(tr[next, :] + alpha0)
    alpha0 = np.full(T, NEG, np.float64)
    alpha0[0] = 0.0
    sc = tr + alpha0[None, :]
    mm = sc.max(axis=1, keepdims=True)
    v = np.log(np.exp(sc - mm).sum(axis=1)) + mm[:, 0]
    vmax = v.max()
    p_init = np.exp(v - vmax)                        # [T]

    bf = ml_dtypes.bfloat16
    w_np = wl.astype(bf)
    pinit_np = np.repeat(p_init[:, None], 32, axis=1).astype(bf)

    r = tr[-1, :]
    r_max = r.max()
    w_last = np.exp(r - r_max)                       # final-row weights [T]
    return w_np, pinit_np, vmax, r_max, w_last


def _host_et(em_core):
    """[BC, S, T] f32 emissions -> [L, 128, COLS] bf16 exp'd + transposed.

    Chain (seg, b) lives at column 32*(seg % (P/2)) + b of tag-block
    seg // (P/2) (block A partitions 0-47, block B 64-111); tick k uses
    E of step seg*L + k. Junk tag-slot rows are 1.0 (finite filler).
    """
    import ml_dtypes

    e = np.exp(em_core.astype(np.float32)) * (2.0 ** C2POW)
    # [b, seg, L, t] -> [blk, s, L, t, b] with seg = blk*(P//2) + s
    e = e.reshape(BC, 2, P // 2, L, T).transpose(1, 2, 3, 4, 0)
    # -> [blk, L, t, s*32 + b]
    e = e.transpose(0, 2, 3, 1, 4).reshape(2, L, T, COLS)
    out = np.ones((L, ROWS, COLS), np.float32)
    out[:, 0:48] = e[0]
    out[:, 64:112] = e[1]
    return out.astype(ml_dtypes.bfloat16)


def _stitch(snap16, snapf, snap2, vmax, r_max, w_last):
    """Per-core host stitch -> [BC] log partition (float64)."""
    def tags(a):  # [112, COLS] -> [T, P, BC] per-chain tag values
        a = np.asarray(a, np.float64)
        return np.concatenate([a[0:48, :], a[64:112, :]], axis=1) \
                 .reshape(T, P, BC)                   # chain = seg*BC + b

    s16 = np.log(np.maximum(tags(snap16).sum(axis=0), 1e-300))   # [P, BC]
    last = tags(snapf)
    sf = np.log(np.maximum(last.sum(axis=0), 1e-300))
    s2 = np.log(np.maximum(tags(snap2).sum(axis=0), 1e-300))

    Lfin = sf[P - 1, :] + (s2[1:, :] - s16[1:, :]).sum(axis=0)

    fin = last[:, -1, :]                              # [T, BC] final-seg state
    d = np.log(np.maximum((w_last[:, None] * fin).sum(axis=0), 1e-300)) \
        - np.log(np.maximum(fin.sum(axis=0), 1e-300))

    return Lfin + d + r_max + vmax - S * C2POW * math.log(2.0)


def _in_maps(emissions, transitions):
    w_np, pinit_np, vmax, r_max, w_last = _host_consts(transitions)
    in_maps = []
    for c in range(NCORES):
        in_maps.append({
            "eet": _host_et(emissions[c * BC:(c + 1) * BC]),
            "wlhs": w_np,
            "pinit": pinit_np,
        })
    return in_maps, (vmax, r_max, w_last)


def kernel(**inputs):
    emissions = np.ascontiguousarray(inputs["emissions"], dtype=np.float32)
    transitions = np.asarray(inputs["transitions"], dtype=np.float32)

    if "nc" not in _CACHE:
        _CACHE["nc"] = _build()
    nc = _CACHE["nc"]

    in_maps, (vmax, r_max, w_last) = _in_maps(emissions, transitions)

    from concourse.bass_utils import run_bass_kernel_spmd
    res = run_bass_kernel_spmd(nc, in_maps, list(range(NCORES))).results

    out = np.empty(B, np.float32)
    for c in range(NCORES):
        r = res[c]
        out[c * BC:(c + 1) * BC] = _stitch(
            r["snap16"], r["snapf"], r["snap2"], vmax, r_max, w_last
        ).astype(np.float32)
    return out


# revision 31
# speedup vs baseline: 1.0042x; 1.0042x over previous
"""CRF log-partition kernel for Trainium2 (8 NeuronCores, data-parallel batch).

Algorithm: the reference forward scan
    alpha' = logsumexp(alpha[None,:] + trans, axis=prev) + emit
is linearized to probability space:
    p' = (M @ p) * E,   M = exp(trans), E = exp(emit) * 2^-7
(the 2^-7 cancels the mean per-step log-growth of ~4.85, so the state
stays in f32/bf16 range with no renormalization; all scale bookkeeping
is recovered on the host from state snapshots).

Each batch item's 2048-step sequence is split into P=64 segments of L=32
steps scanned in parallel (products of positive matrices converge to
rank-1, so each segment's output direction is independent of its init;
scales are fixed up by an 8-tick prefix-correction pass seeded with the
previous segment's final state). Per core: 32 batch x 64 segments = 2048
chains laid out as X[128, 1024] bf16 — tag-block A (chains 0-1023) on
partitions 0-47, block B (chains 1024-2047) on partitions 64-111 (junk
rows are killed by zero rows of the padded [112,128] lhsT).

E = exp(emissions)*2^-7 is precomputed on the HOST in bf16, already
transposed to the on-chip layout [tick, tag-slot(128), chain(1024)], so
the device pipeline is just: DMA -> per-tick (PE matmul, DVE/Pool
elementwise multiply). No on-device exp, no PE transposes, no
PSUM->SBUF staging copies. Per tick the 1024 columns are split into
independent column-groups, each its own serial matmul->multiply chain
(staggered to hide the PE->DVE/Pool semaphore+PSUM-access latency);
groups are assigned to DVE and Pool to balance engine busy time.

Host stitches per-(batch,segment) log-scales in float64 from three bf16
snapshots per core: pass-1 state at tick PREFIX, pass-1 final, pass-2
final.

mask does not affect the forward value (m*x + (1-m)*x == x) and is ignored.
"""

import math

import numpy as np

B, S, T = 256, 2048, 48
NEG = -10000.0
NCORES = 8
BC = B // NCORES          # batch per core = 32
P = 64                    # segments per batch item
L = S // P                # ticks per segment = 32
PREFIX = 2                # prefix-correction ticks
C2POW = -7                # constant rescale folded into E
COLS = BC * P // 2        # chains per block (columns of X) = 1024
ROWS = 112                # meaningful partition rows (blocks at 0-47 / 64-111)
XROWS = 128               # physical tile rows
TPG = 1                   # ticks per resident eT tile (one DMA each)
QBUFS = 2                 # PSUM q bufs per column-group
XBUFS = 8
# per-tick column-groups: (engine, col0, col1); "v"=DVE, "p"=Pool
# "v": DVE multiplies straight from PSUM; "p": ACT copies q to SBUF bf16
# (GPSIMD cannot read PSUM), then Pool multiplies all-SBUF.
MGROUPS = (("v", 0, 512), ("v", 512, 1024))
PSKEW = 2                 # p-group issue lag (ticks) to avoid PE head-of-line

_CACHE = {}


def _build(**cfg):
    g = globals()
    saved = {k: g[k] for k in cfg}
    g.update(cfg)
    try:
        return _build_inner()
    finally:
        g.update(saved)


def _build_inner():
    from contextlib import ExitStack

    import concourse.bacc as bacc
    import concourse.bass as bass
    import concourse.mybir as mybir
    import concourse.tile as tile

    f32 = mybir.dt.float32
    bf16 = mybir.dt.bfloat16

    nc = bacc.Bacc(None, target_bir_lowering=False)

    NTILE = L // TPG
    eet_d = nc.dram_tensor("eet", [L, ROWS, COLS], bf16, kind="ExternalInput")
    w_d = nc.dram_tensor("wlhs", [ROWS, XROWS], bf16, kind="ExternalInput")
    pinit_d = nc.dram_tensor("pinit", [48, 32], bf16, kind="ExternalInput")
    snap16_d = nc.dram_tensor("snap16", [ROWS, COLS], bf16, kind="ExternalOutput")
    snapf_d = nc.dram_tensor("snapf", [ROWS, COLS], bf16, kind="ExternalOutput")
    snap2_d = nc.dram_tensor("snap2", [ROWS, COLS], bf16, kind="ExternalOutput")

    with tile.TileContext(nc) as tc:
        with ExitStack() as ctx:
            consts = ctx.enter_context(tc.tile_pool(name="consts", bufs=1))
            epool = ctx.enter_context(tc.tile_pool(name="epool", bufs=NTILE))
            xpool = ctx.enter_context(tc.tile_pool(name="xpool", bufs=XBUFS))
            qpool = ctx.enter_context(
                tc.tile_pool(name="qpool", bufs=QBUFS, space=bass.MemorySpace.PSUM))
            qbpool = ctx.enter_context(tc.tile_pool(name="qbpool", bufs=QBUFS))

            w_sb = consts.tile([ROWS, XROWS], bf16, tag="w")

            NG = len(MGROUPS)
            GW = [c1 - c0 for (_, c0, c1) in MGROUPS]
            lag = [0 if kind == "v" else PSKEW for (kind, _, _) in MGROUPS]

            xg = []
            for gi in range(NG):
                xt = xpool.tile([XROWS, GW[gi]], bf16, tag=f"x{gi}",
                                name=f"x{gi}")
                nc.gpsimd.memset(xt[:], 1.0)
                xg.append(xt)

            # tiny pinit/w loads lead their rings so tick 0 isn't gated by
            # the bulk eT stream; eT tile 0 goes next on the sync ring
            nc.sync.dma_start(xg[0][0:48, 0:32], pinit_d[:])  # cols 0-31 g0
            nc.scalar.dma_start(w_sb[:], w_d[:])
            etiles = []
            for i in range(NTILE):
                et = epool.tile([XROWS, TPG * COLS], bf16, tag="e",
                                name=f"e{i}")
                src = eet_d[i * TPG:(i + 1) * TPG].rearrange("t p c -> p t c")
                dma_eng = nc.scalar if i % 2 else nc.sync
                dma_eng.dma_start(et[0:ROWS, :], src)
                etiles.append(et)

            def tick_g(gi, x_in, k, matmul):
                kind, c0, c1 = MGROUPS[gi]
                base = (k % TPG) * COLS
                esl = etiles[k // TPG][:, base + c0:base + c1]
                x_out = xpool.tile([XROWS, GW[gi]], bf16, tag=f"x{gi}",
                                   name=f"x{gi}_{k}")
                eng = nc.vector if kind == "v" else nc.gpsimd
                if not matmul:
                    eng.tensor_mul(x_out[:], x_in[:], esl)
                    return x_out
                q = qpool.tile([XROWS, GW[gi]], f32, tag=f"q{gi}",
                               name=f"q{gi}")
                nc.tensor.matmul(q[:], w_sb[:], x_in[0:ROWS, :])
                if kind == "v":
                    eng.tensor_mul(x_out[:], q[:], esl)
                else:
                    qb = qbpool.tile([XROWS, GW[gi]], bf16,
                                     tag=f"qb{gi}", name=f"qb{gi}")
                    nc.scalar.activation(
                        qb[:], q[:], mybir.ActivationFunctionType.Copy)
                    eng.tensor_mul(x_out[:], qb[:], esl)
                return x_out

            # pass 1; p-groups lag PSKEW ticks in issue order so their
            # slower chains never block the PE queue head for the v-groups
            for r in range(L + PSKEW):
                for gi, (kind, c0, c1) in enumerate(MGROUPS):
                    k = r - lag[gi]
                    if not 0 <= k < L:
                        continue
                    xg[gi] = tick_g(gi, xg[gi], k, matmul=(k > 0))
                    if k + 1 == PREFIX:
                        nc.sync.dma_start(snap16_d[0:ROWS, c0:c1],
                                          xg[gi][0:ROWS, :])
                    if k + 1 == L:
                        nc.sync.dma_start(snapf_d[0:ROWS, c0:c1],
                                          xg[gi][0:ROWS, :])

            # pass 2: init = pass-1 finals shifted by one 32-col segment slot
            x2g = []
            for gi in range(NG):
                w = GW[gi]
                x2 = xpool.tile([XROWS, w], bf16, tag=f"x{gi}",
                                name=f"x2_{gi}")
                if gi == 0:
                    nc.gpsimd.memset(x2[:, 0:32], 1.0)     # seg-0 slot unused
                    # block-B first seg init <- block-A last seg (A rows of
                    # the very last column slot, partition-shifted via DMA)
                    nc.sync.dma_start(
                        x2[64:112, 0:32], xg[NG - 1][0:48, GW[NG - 1] - 32:])
                else:
                    nc.vector.tensor_copy(
                        x2[:, 0:32], xg[gi - 1][:, GW[gi - 1] - 32:])
                nc.vector.tensor_copy(x2[:, 32:w], xg[gi][:, 0:w - 32])
                x2g.append(x2)
            for r in range(PREFIX + PSKEW):
                for gi, (kind, c0, c1) in enumerate(MGROUPS):
                    k = r - lag[gi]
                    if not 0 <= k < PREFIX:
                        continue
                    x2g[gi] = tick_g(gi, x2g[gi], k, matmul=True)
                    if k + 1 == PREFIX:
                        nc.sync.dma_start(snap2_d[0:ROWS, c0:c1],
                                          x2g[gi][0:ROWS, :])

    nc.compile()
    return nc


def _host_consts(transitions):
    """W lhsT, p_init (analytic first log-step), stitch constants."""
    import ml_dtypes

    tr = transitions.astype(np.float64)
    M = np.exp(tr)                                   # M[next, prev]
    wl = np.zeros((ROWS, XROWS), np.float64)
    wl[0:48, 0:48] = M.T                             # lhsT[k, m] = M[m, k]
    wl[64:112, 64:112] = M.T

    # analytic first step: v[next] = logsumexp_prev(tr[next, :] + alpha0)
    alpha0 = np.full(T, NEG, np.float64)
    alpha0[0] = 0.0
    sc = tr + alpha0[None, :]
    mm = sc.max(axis=1, keepdims=True)
    v = np.log(np.exp(sc - mm).sum(axis=1)) + mm[:, 0]
    vmax = v.max()
    p_init = np.exp(v - vmax)                        # [T]

    bf = ml_dtypes.bfloat16
    w_np = wl.astype(bf)
    pinit_np = np.repeat(p_init[:, None], 32, axis=1).astype(bf)

    r = tr[-1, :]
    r_max = r.max()
    w_last = np.exp(r - r_max)                       # final-row weights [T]
    return w_np, pinit_np, vmax, r_max, w_last


def _host_et(em_core):
    """[BC, S, T] f32 emissions -> [L, 128, COLS] bf16 exp'd + transposed.

    Chain (seg, b) lives at column 32*(seg % (P/2)) + b of tag-block
    seg // (P/2) (block A partitions 0-47, block B 64-111); tick k uses
    E of step seg*L + k. Junk tag-slot rows are 1.0 (finite filler).
    """
    import ml_dtypes

    e = np.exp(em_core.astype(np.float32)) * (2.0 ** C2POW)
    # [b, seg, L, t] -> [blk, s, L, t, b] with seg = blk*(P//2) + s
    e = e.reshape(BC, 2, P // 2, L, T).transpose(1, 2, 3, 4, 0)
    # -> [blk, L, t, s*32 + b]
    e = e.transpose(0, 2, 3, 1, 4).reshape(2, L, T, COLS)
    out = np.ones((L, ROWS, COLS), np.float32)
    out[:, 0:48] = e[0]
    out[:, 64:112] = e[1]
    return out.astype(ml_dtypes.bfloat16)


def _stitch(snap16, snapf, snap2, vmax, r_max, w_last):
    """Per-core host stitch -> [BC] log partition (float64)."""
    def tags(a):  # [112, COLS] -> [T, P, BC] per-chain tag values
        a = np.asarray(a, np.float64)
        return np.concatenate([a[0:48, :], a[64:112, :]], axis=1) \
                 .reshape(T, P, BC)                   # chain = seg*BC + b

    s16 = np.log(np.maximum(tags(snap16).sum(axis=0), 1e-300))   # [P, BC]
    last = tags(snapf)
    sf = np.log(np.maximum(last.sum(axis=0), 1e-300))
    s2 = np.log(np.maximum(tags(snap2).sum(axis=0), 1e-300))

    Lfin = sf[P - 1, :] + (s2[1:, :] - s16[1:, :]).sum(axis=0)

    fin = last[:, -1, :]                              # [T, BC] final-seg state
    d = np.log(np.maximum((w_last[:, None] * fin).sum(axis=0), 1e-300)) \
        - np.log(np.maximum(fin.sum(axis=0), 1e-300))

    return Lfin + d + r_max + vmax - S * C2POW * math.log(2.0)


def _in_maps(emissions, transitions):
    w_np, pinit_np, vmax, r_max, w_last = _host_consts(transitions)
    in_maps = []
    for c in range(NCORES):
        in_maps.append({
            "eet": _host_et(emissions[c * BC:(c + 1) * BC]),
            "wlhs": w_np,
            "pinit": pinit_np,
        })
    return in_maps, (vmax, r_max, w_last)


def kernel(**inputs):
    emissions = np.ascontiguousarray(inputs["emissions"], dtype=np.float32)
    transitions = np.asarray(inputs["transitions"], dtype=np.float32)

    if "nc" not in _CACHE:
        _CACHE["nc"] = _build()
    nc = _CACHE["nc"]

    in_maps, (vmax, r_max, w_last) = _in_maps(emissions, transitions)

    from concourse.bass_utils import run_bass_kernel_spmd
    res = run_bass_kernel_spmd(nc, in_maps, list(range(NCORES))).results

    out = np.empty(B, np.float32)
    for c in range(NCORES):
        r = res[c]
        out[c * BC:(c + 1) * BC] = _stitch(
            r["snap16"], r["snapf"], r["snap2"], vmax, r_max, w_last
        ).astype(np.float32)
    return out


# revision 37
# speedup vs baseline: 1.0251x; 1.0208x over previous
"""CRF log-partition kernel for Trainium2 (8 NeuronCores, data-parallel batch).

Algorithm: the reference forward scan
    alpha' = logsumexp(alpha[None,:] + trans, axis=prev) + emit
is linearized to probability space:
    p' = (M @ p) * E,   M = exp(trans), E = exp(emit) * 2^-7
(the 2^-7 cancels the mean per-step log-growth of ~4.85, so the state
stays in f32/bf16 range with no renormalization; all scale bookkeeping
is recovered on the host from state snapshots).

Each batch item's 2048-step sequence is split into P=64 segments of L=32
steps scanned in parallel (products of positive matrices converge to
rank-1, so each segment's output direction is independent of its init;
scales are fixed up by an 8-tick prefix-correction pass seeded with the
previous segment's final state). Per core: 32 batch x 64 segments = 2048
chains laid out as X[128, 1024] bf16 — tag-block A (chains 0-1023) on
partitions 0-47, block B (chains 1024-2047) on partitions 64-111 (junk
rows are killed by zero rows of the padded [112,128] lhsT).

E = exp(emissions)*2^-7 is precomputed on the HOST in bf16, already
transposed to the on-chip layout [tick, tag-slot(128), chain(1024)], so
the device pipeline is just: DMA -> per-tick (PE matmul, DVE/Pool
elementwise multiply). No on-device exp, no PE transposes, no
PSUM->SBUF staging copies. Per tick the 1024 columns are split into
independent column-groups, each its own serial matmul->multiply chain
(staggered to hide the PE->DVE/Pool semaphore+PSUM-access latency);
groups are assigned to DVE and Pool to balance engine busy time.

Host stitches per-(batch,segment) log-scales in float64 from three bf16
snapshots per core: pass-1 state at tick PREFIX, pass-1 final, pass-2
final.

mask does not affect the forward value (m*x + (1-m)*x == x) and is ignored.
"""

import math

import numpy as np

B, S, T = 256, 2048, 48
NEG = -10000.0
NCORES = 8
BC = B // NCORES          # batch per core = 32
P = 64                    # segments per batch item
L = S // P                # ticks per segment = 32
PREFIX = 1                # prefix-correction ticks
C2POW = -7                # constant rescale folded into E
COLS = BC * P // 2        # chains per block (columns of X) = 1024
ROWS = 112                # meaningful partition rows (blocks at 0-47 / 64-111)
XROWS = 128               # physical tile rows
TPG = 1                   # ticks per resident eT tile (one DMA each)
QBUFS = 2                 # PSUM q bufs per column-group
XBUFS = 8
# per-tick column-groups: (engine, col0, col1); "v"=DVE, "p"=Pool
# "v": DVE multiplies straight from PSUM; "p": ACT copies q to SBUF bf16
# (GPSIMD cannot read PSUM), then Pool multiplies all-SBUF.
MGROUPS = (("v", 0, 512), ("v", 512, 1024))
PSKEW = 2                 # p-group issue lag (ticks) to avoid PE head-of-line

_CACHE = {}


def _build(**cfg):
    g = globals()
    saved = {k: g[k] for k in cfg}
    g.update(cfg)
    try:
        return _build_inner()
    finally:
        g.update(saved)


def _build_inner():
    from contextlib import ExitStack

    import concourse.bacc as bacc
    import concourse.bass as bass
    import concourse.mybir as mybir
    import concourse.tile as tile

    f32 = mybir.dt.float32
    bf16 = mybir.dt.bfloat16

    nc = bacc.Bacc(None, target_bir_lowering=False)

    NTILE = L // TPG
    eet_d = nc.dram_tensor("eet", [L, ROWS, COLS], bf16, kind="ExternalInput")
    w_d = nc.dram_tensor("wlhs", [ROWS, XROWS], bf16, kind="ExternalInput")
    pinit_d = nc.dram_tensor("pinit", [48, 32], bf16, kind="ExternalInput")
    snap16_d = nc.dram_tensor("snap16", [ROWS, COLS], bf16, kind="ExternalOutput")
    snapf_d = nc.dram_tensor("snapf", [ROWS, COLS], bf16, kind="ExternalOutput")
    snap2_d = nc.dram_tensor("snap2", [ROWS, COLS], bf16, kind="ExternalOutput")

    with tile.TileContext(nc) as tc:
        with ExitStack() as ctx:
            consts = ctx.enter_context(tc.tile_pool(name="consts", bufs=1))
            epool = ctx.enter_context(tc.tile_pool(name="epool", bufs=NTILE))
            xpool = ctx.enter_context(tc.tile_pool(name="xpool", bufs=XBUFS))
            qpool = ctx.enter_context(
                tc.tile_pool(name="qpool", bufs=QBUFS, space=bass.MemorySpace.PSUM))
            qbpool = ctx.enter_context(tc.tile_pool(name="qbpool", bufs=QBUFS))

            w_sb = consts.tile([ROWS, XROWS], bf16, tag="w")

            NG = len(MGROUPS)
            GW = [c1 - c0 for (_, c0, c1) in MGROUPS]
            lag = [0 if kind == "v" else PSKEW for (kind, _, _) in MGROUPS]

            pv = xpool.tile([XROWS, 32], bf16, tag="pv", bufs=1)
            nc.gpsimd.memset(pv[:], 1.0)

            # tiny pinit/w loads lead their rings so tick 1 isn't gated by
            # the bulk eT stream; eT tile 0 goes next on the sync ring
            nc.sync.dma_start(pv[0:48, :], pinit_d[:])
            nc.scalar.dma_start(w_sb[:], w_d[:])
            etiles = []
            for i in range(NTILE):
                et = epool.tile([XROWS, TPG * COLS], bf16, tag="e",
                                name=f"e{i}")
                src = eet_d[i * TPG:(i + 1) * TPG].rearrange("t p c -> p t c")
                dma_eng = nc.scalar if i % 2 else nc.sync
                dma_eng.dma_start(et[0:ROWS, :], src)
                etiles.append(et)

            # tick 0 is x = init ⊙ E_0 and init is ones outside cols 0-31,
            # so patch pinit into eT tile 0 in place and start the scan at
            # tick 1 reading E_0 as the state. The patch corrupts only the
            # seg-0 columns of E_0, which pass 2 discards in the stitch.
            nc.vector.tensor_mul(etiles[0][:, 0:32], etiles[0][:, 0:32],
                                 pv[:])
            xg = [etiles[0][:, c0:c1] for (_, c0, c1) in MGROUPS]
            if PREFIX == 1:
                # state after tick 0 IS the patched E_0 tile (seg-0 cols are
                # corrupted by the patch but the stitch discards seg 0)
                nc.sync.dma_start(snap16_d[:], etiles[0][0:ROWS, 0:COLS])

            def tick_g(gi, x_in, k, matmul):
                kind, c0, c1 = MGROUPS[gi]
                base = (k % TPG) * COLS
                esl = etiles[k // TPG][:, base + c0:base + c1]
                x_out = xpool.tile([XROWS, GW[gi]], bf16, tag=f"x{gi}",
                                   name=f"x{gi}_{k}")
                eng = nc.vector if kind == "v" else nc.gpsimd
                if not matmul:
                    eng.tensor_mul(x_out[:], x_in[:], esl)
                    return x_out
                q = qpool.tile([XROWS, GW[gi]], f32, tag=f"q{gi}",
                               name=f"q{gi}")
                nc.tensor.matmul(q[:], w_sb[:], x_in[0:ROWS, :])
                if kind == "v":
                    eng.tensor_mul(x_out[:], q[:], esl)
                else:
                    qb = qbpool.tile([XROWS, GW[gi]], bf16,
                                     tag=f"qb{gi}", name=f"qb{gi}")
                    nc.scalar.activation(
                        qb[:], q[:], mybir.ActivationFunctionType.Copy)
                    eng.tensor_mul(x_out[:], qb[:], esl)
                return x_out

            # pass 1; p-groups lag PSKEW ticks in issue order so their
            # slower chains never block the PE queue head for the v-groups
            for r in range(L + PSKEW):
                for gi, (kind, c0, c1) in enumerate(MGROUPS):
                    k = r - lag[gi]
                    if not 1 <= k < L:
                        continue
                    xg[gi] = tick_g(gi, xg[gi], k, matmul=True)
                    if k + 1 == PREFIX:
                        nc.sync.dma_start(snap16_d[0:ROWS, c0:c1],
                                          xg[gi][0:ROWS, :])
                    if k + 1 == L:
                        nc.sync.dma_start(snapf_d[0:ROWS, c0:c1],
                                          xg[gi][0:ROWS, :])

            # pass 2: init = pass-1 finals shifted by one 32-col segment slot
            x2g = []
            for gi in range(NG):
                w = GW[gi]
                x2 = xpool.tile([XROWS, w], bf16, tag=f"x{gi}",
                                name=f"x2_{gi}")
                if gi == 0:
                    nc.gpsimd.memset(x2[:, 0:32], 1.0)     # seg-0 slot unused
                    # block-B first seg init <- block-A last seg (A rows of
                    # the very last column slot, partition-shifted via DMA)
                    nc.sync.dma_start(
                        x2[64:112, 0:32], xg[NG - 1][0:48, GW[NG - 1] - 32:])
                else:
                    nc.vector.tensor_copy(
                        x2[:, 0:32], xg[gi - 1][:, GW[gi - 1] - 32:])
                nc.vector.tensor_copy(x2[:, 32:w], xg[gi][:, 0:w - 32])
                x2g.append(x2)
            for r in range(PREFIX + PSKEW):
                for gi, (kind, c0, c1) in enumerate(MGROUPS):
                    k = r - lag[gi]
                    if not 0 <= k < PREFIX:
                        continue
                    x2g[gi] = tick_g(gi, x2g[gi], k, matmul=True)
                    if k + 1 == PREFIX:
                        nc.sync.dma_start(snap2_d[0:ROWS, c0:c1],
                                          x2g[gi][0:ROWS, :])

    nc.compile()
    return nc


def _host_consts(transitions):
    """W lhsT, p_init (analytic first log-step), stitch constants."""
    import ml_dtypes

    tr = transitions.astype(np.float64)
    M = np.exp(tr)                                   # M[next, prev]
    wl = np.zeros((ROWS, XROWS), np.float64)
    wl[0:48, 0:48] = M.T                             # lhsT[k, m] = M[m, k]
    wl[64:112, 64:112] = M.T

    # analytic first step: v[next] = logsumexp_prev(tr[next, :] + alpha0)
    alpha0 = np.full(T, NEG, np.float64)
    alpha0[0] = 0.0
    sc = tr + alpha0[None, :]
    mm = sc.max(axis=1, keepdims=True)
    v = np.log(np.exp(sc - mm).sum(axis=1)) + mm[:, 0]
    vmax = v.max()
    p_init = np.exp(v - vmax)                        # [T]

    bf = ml_dtypes.bfloat16
    w_np = wl.astype(bf)
    pinit_np = np.repeat(p_init[:, None], 32, axis=1).astype(bf)

    r = tr[-1, :]
    r_max = r.max()
    w_last = np.exp(r - r_max)                       # final-row weights [T]
    return w_np, pinit_np, vmax, r_max, w_last


def _host_et(em_core):
    """[BC, S, T] f32 emissions -> [L, 128, COLS] bf16 exp'd + transposed.

    Chain (seg, b) lives at column 32*(seg % (P/2)) + b of tag-block
    seg // (P/2) (block A partitions 0-47, block B 64-111); tick k uses
    E of step seg*L + k. Junk tag-slot rows are 1.0 (finite filler).
    """
    import ml_dtypes

    e = np.exp(em_core.astype(np.float32)) * (2.0 ** C2POW)
    # [b, seg, L, t] -> [blk, s, L, t, b] with seg = blk*(P//2) + s
    e = e.reshape(BC, 2, P // 2, L, T).transpose(1, 2, 3, 4, 0)
    # -> [blk, L, t, s*32 + b]
    e = e.transpose(0, 2, 3, 1, 4).reshape(2, L, T, COLS)
    out = np.ones((L, ROWS, COLS), np.float32)
    out[:, 0:48] = e[0]
    out[:, 64:112] = e[1]
    return out.astype(ml_dtypes.bfloat16)


def _stitch(snap16, snapf, snap2, vmax, r_max, w_last):
    """Per-core host stitch -> [BC] log partition (float64)."""
    def tags(a):  # [112, COLS] -> [T, P, BC] per-chain tag values
        a = np.asarray(a, np.float64)
        return np.concatenate([a[0:48, :], a[64:112, :]], axis=1) \
                 .reshape(T, P, BC)                   # chain = seg*BC + b

    s16 = np.log(np.maximum(tags(snap16).sum(axis=0), 1e-300))   # [P, BC]
    last = tags(snapf)
    sf = np.log(np.maximum(last.sum(axis=0), 1e-300))
    s2 = np.log(np.maximum(tags(snap2).sum(axis=0), 1e-300))

    Lfin = sf[P - 1, :] + (s2[1:, :] - s16[1:, :]).sum(axis=0)

    fin = last[:, -1, :]                              # [T, BC] final-seg state
    d = np.log(np.maximum((w_last[:, None] * fin).sum(axis=0), 1e-300)) \
        - np.log(np.maximum(fin.sum(axis=0), 1e-300))

    return Lfin + d + r_max + vmax - S * C2POW * math.log(2.0)


def _in_maps(emissions, transitions):
    w_np, pinit_np, vmax, r_max, w_last = _host_consts(transitions)
    in_maps = []
    for c in range(NCORES):
        in_maps.append({
            "eet": _host_et(emissions[c * BC:(c + 1) * BC]),
            "wlhs": w_np,
            "pinit": pinit_np,
        })
    return in_maps, (vmax, r_max, w_last)


def kernel(**inputs):
    emissions = np.ascontiguousarray(inputs["emissions"], dtype=np.float32)
    transitions = np.asarray(inputs["transitions"], dtype=np.float32)

    if "nc" not in _CACHE:
        _CACHE["nc"] = _build()
    nc = _CACHE["nc"]

    in_maps, (vmax, r_max, w_last) = _in_maps(emissions, transitions)

    from concourse.bass_utils import run_bass_kernel_spmd
    res = run_bass_kernel_spmd(nc, in_maps, list(range(NCORES))).results

    out = np.empty(B, np.float32)
    for c in range(NCORES):
        r = res[c]
        out[c * BC:(c + 1) * BC] = _stitch(
            r["snap16"], r["snapf"], r["snap2"], vmax, r_max, w_last
        ).astype(np.float32)
    return out
